# revision 34
# baseline (speedup 1.0000x reference)
"""Trainium2 Bass kernel for nn_EquivariantBiLinear (o-sharded T-formulation).

Math (per batch row b):
    W_proj[b, j] = sum_nu W_g[mu, nu] * x[b, bid_g[nu*r+rho]]   (j = off_g + mu*r + rho)
    out[b, o]    = 0.1 * sum_i W_proj[b, q[o*256+i]] * x[b, i]  (q = W_invperm)

W_proj is linear in x:  W_proj[b,:] = M x[b,:],
    M[j, i'] = sum_nu W_g[mu, nu] [bid_g[nu*r+rho] == i'].
With T[(o,i), i'] = M[q[o*256+i], i']:
    out[b, o] = 0.1 * sum_i x[b,i] * (T[(o,i),:] . x[b,:]).

Sharding: each core owns 32 output columns o (8192 T-rows).  Per core:
  1. T-build: per (g,rho) GEMMs T[k, i'] = W~^T S (S = one-hot scatter of
     bid, W~ = host-gathered W columns) in unsorted (g,rho)-blocked order;
     evacuate fp16 and stream to a DRAM staging table (chunk-major rows).
  2. Sort: 64 indirect row-gathers (DGE) pull the 8192 rows back in
     (o,i)-sorted order; same SWDGE queue as the write-out so ordering is
     FIFO.  PE transposes flip each (128k x 256i') tile into T^T.
  3. GEMM1: Z = X @ T^T in PSUM (the irreducible 8.6 GFLOP/core).
  4. Combine: out[b,o] = sum_i 0.1*Z[b,(o,i)]*x[b,i] via the fused DVE
     affine_mul_reduce, with Scalar evacuating most Z tiles to SBUF fp16.
"""

import sys

if "/opt/trn_rl_repo" not in sys.path:
    sys.path.insert(0, "/opt/trn_rl_repo")

from contextlib import ExitStack

import numpy as np

import concourse.bacc as bacc
import concourse.bass as bass
import concourse.mybir as mybir
import concourse.tile as tile
from concourse.bass import IndirectOffsetOnAxis
from concourse.bass_utils import run_bass_kernel_spmd
from concourse.masks import make_identity

GROUPS = [(512, 1, 16384), (256, 4, 4096), (128, 16, 1024), (64, 64, 256)]
OFF = [0, 16384, 32768, 49152]
X = 256
B = 2048
NCORES = 8
OPC = X // NCORES  # 32 output cols per core
KPC = OPC * X      # 8192 sorted T-cols per core

F32 = mybir.dt.float32
FP16 = mybir.dt.float16
I32 = mybir.dt.int32

KT_N = [4, 2, 1, 1]  # contraction 128-tiles per group (g3 rho-pair-stacked)
STAGE_SLOTS = 16     # 128-row chunks per DRAM write-out


def _host_prep(W0, W1, W2, W3, bid0, bid1, bid2, bid3, W_invperm):
    """Index layout + per-core weight gathers (no arithmetic on data)."""
    q = np.asarray(W_invperm).astype(np.int64)
    bids = [np.asarray(b).astype(np.int64) for b in (bid0, bid1, bid2, bid3)]
    Ws = [np.asarray(W) for W in (W0, W1, W2, W3)]

    o_all = np.arange(X * X) // X
    core_all = o_all // OPC
    g_all = np.searchsorted([16384, 32768, 49152, 65536], q, side="right")
    jj = q - np.asarray(OFF)[g_all]
    r_all = np.asarray([1, 4, 16, 64])[g_all]
    mu_all = jj // r_all
    rho_all = jj % r_all

    lists = {}
    for c in range(NCORES):
        sel = np.nonzero(core_all == c)[0]
        js = (o_all[sel] % OPC) * X + (sel % X)
        for g in range(4):
            m = g_all[sel] == g
            for rho in range(GROUPS[g][1]):
                mm = m & (rho_all[sel] == rho)
                lists[(c, g, rho)] = (mu_all[sel][mm], js[mm])

    # block layout: (g, rho_lo, nrho, P, boff, goffg); g3 pairs two rhos
    blocks = []
    off = 0
    goff = [0, 0, 0, 0]
    for g in range(4):
        n, r, m = GROUPS[g]
        if g < 3:
            for rho in range(r):
                P = max(len(lists[(c, g, rho)][0]) for c in range(NCORES))
                P = -(-P // 4) * 4
                blocks.append((g, rho, 1, P, off, goff[g]))
                off += P
                goff[g] += P
        else:
            for pair in range(r // 2):
                Ph = max(len(lists[(c, g, 2 * pair + s)][0])
                         for c in range(NCORES) for s in (0, 1))
                Ph = -(-Ph // 4) * 4
                blocks.append((g, 2 * pair, 2, 2 * Ph, off, goff[g]))
                off += 2 * Ph
                goff[g] += 2 * Ph
    PTOT = off
    gtot = tuple(goff)

    # chunk table: 128-col pieces of each block, in order -> td2 row blocks
    chunks = []  # (g, rho, nrho, P, boff, goffg, coff, cw, jglobal)
    for (g, rho, nrho, P, boff, goffg) in blocks:
        for coff in range(0, P, 128):
            cw = min(128, P - coff)
            chunks.append((g, rho, nrho, P, boff, goffg, coff, cw, len(chunks)))
    NCH = len(chunks)

    # bid columns for on-device one-hot build: 60 cols =
    # g0: 4 kt | g1: 8 (kt,rho) | g2: 16 rho | g3: 32 stacked pairs
    bidt = np.zeros((128, 60), np.float32)
    col = 0
    for kt in range(4):
        bidt[:, col] = bids[0][128 * kt:128 * (kt + 1)]
        col += 1
    for rho in range(4):
        for kt in range(2):
            b = bids[1].reshape(256, 4)[:, rho]
            bidt[:, col] = b[128 * kt:128 * (kt + 1)]
            col += 1
    for rho in range(16):
        bidt[:, col] = bids[2].reshape(128, 16)[:, rho]
        col += 1
    b3 = bids[3].reshape(64, 64)
    for pair in range(32):
        bidt[0:64, col] = b3[:, 2 * pair]
        bidt[64:128, col] = b3[:, 2 * pair + 1]
        col += 1
    assert col == 60
    iota = np.ascontiguousarray(
        np.tile(np.arange(X, dtype=np.float16), (128, 1)))
    s_arrs = [np.ascontiguousarray(bidt), iota]

    # per-core gathered W columns + sorted-row indices into td2
    w_percore = []
    idx_percore = []
    # block rank -> (chunk j, row) map, same for all cores
    blk_chunkbase = {}
    for ch in chunks:
        (g, rho, nrho, P, boff, goffg, coff, cw, j) = ch
        blk_chunkbase.setdefault(boff, []).append((coff, j))
    for c in range(NCORES):
        wt = [np.zeros((512, gtot[0]), np.float16),
              np.zeros((256, gtot[1]), np.float16),
              np.zeros((128, gtot[2]), np.float16),
              np.zeros((128, gtot[3]), np.float16)]
        td2row = np.zeros(KPC, np.int64)  # sorted slot -> td2 row
        for (g, rho, nrho, P, boff, goffg) in blocks:
            n, r, m = GROUPS[g]
            Wg2 = Ws[g].reshape(m, n)
            cmap = dict(blk_chunkbase[boff])  # coff -> j

            def rank_to_row(rank):
                return 128 * cmap[(rank // 128) * 128] + rank % 128

            if nrho == 1:
                mus, js = lists[(c, g, rho)]
                cnt = len(mus)
                wt[g][:, goffg:goffg + cnt] = Wg2[mus, :].T.astype(np.float16)
                td2row[js] = [rank_to_row(t) for t in range(cnt)]
            else:
                half = P // 2
                for s in (0, 1):
                    mus, js = lists[(c, g, rho + s)]
                    cnt = len(mus)
                    lo, hi = (0, 64) if s == 0 else (64, 128)
                    wt[g][lo:hi, goffg + s * half:goffg + s * half + cnt] = \
                        Wg2[mus, :].T.astype(np.float16)
                    td2row[js] = [rank_to_row(s * half + t) for t in range(cnt)]
        w_percore.append([np.ascontiguousarray(w) for w in wt])
        idxg = np.ascontiguousarray(
            td2row.reshape(64, 128).T.astype(np.int32))  # [p, c]
        idx_percore.append(idxg)

    layout = (tuple(blocks), tuple(ch[:8] for ch in chunks), PTOT, gtot, NCH)
    return layout, s_arrs, w_percore, idx_percore


def _build_nc(layout):
    blocks, chunks_t, PTOT, gtot, NCH = layout
    chunks = [tuple(ch) + (j,) for j, ch in enumerate(chunks_t)]
    nc = bacc.Bacc("TRN2", target_bir_lowering=False, debug=False,
                   num_devices=NCORES)

    bid_d = nc.dram_tensor("bidt", [128, 60], F32, kind="ExternalInput")
    iota_d = nc.dram_tensor("iota", [128, 256], FP16, kind="ExternalInput")
    w_d = [nc.dram_tensor("w0", [512, gtot[0]], FP16, kind="ExternalInput"),
           nc.dram_tensor("w1", [256, gtot[1]], FP16, kind="ExternalInput"),
           nc.dram_tensor("w2", [128, gtot[2]], FP16, kind="ExternalInput"),
           nc.dram_tensor("w3", [128, gtot[3]], FP16, kind="ExternalInput")]
    xt_d = nc.dram_tensor("xt", [256, 2048], FP16, kind="ExternalInput")
    xb_d = nc.dram_tensor("xb", [2048, 256], FP16, kind="ExternalInput")
    idx_d = nc.dram_tensor("idx", [128, 64], I32, kind="ExternalInput")
    td_d = nc.dram_tensor("td", [128 * NCH, 256], FP16, kind="Internal")
    out_d = nc.dram_tensor("out", [2048, 32], F32, kind="ExternalOutput")

    with tile.TileContext(nc) as tc, ExitStack() as ctx:
        const = ctx.enter_context(tc.tile_pool(name="const", bufs=1))

        ident = const.tile([128, 128], FP16)
        make_identity(nc, ident[:])
        idxt = const.tile([128, 64], I32)
        nc.sync.dma_start(idxt[:], idx_d[:])
        xtt = [const.tile([128, 2048], FP16, tag=f"xtt{ih}", name=f"xtt{ih}")
               for ih in range(2)]
        for ih in range(2):
            nc.sync.dma_start(xtt[ih][:], xt_d[128 * ih:128 * (ih + 1), :])
        xbt = [const.tile([128, 256], FP16, tag=f"xb{bt}", name=f"xb{bt}")
               for bt in range(16)]
        for bt in range(16):
            nc.sync.dma_start(xbt[bt][:], xb_d[128 * bt:128 * (bt + 1), :])
        Tb = [const.tile([128, KPC], FP16, tag=f"tb{ih}", name=f"tb{ih}")
              for ih in range(2)]
        outsb = [const.tile([128, 32], F32, tag=f"os{bt}", name=f"os{bt}")
                 for bt in range(16)]

        ev_state = {"n": 0}

        def alt_copy(dst, src):
            if ev_state["n"] % 2 == 0:
                nc.vector.tensor_copy(dst, src)
            else:
                nc.scalar.copy(dst, src)
            ev_state["n"] += 1

        # ---- phase 1: T-build -> staged DRAM write-out (gpsimd queue) ----
        with tc.tile_pool(name="wsp", bufs=1) as wsp, \
             tc.tile_pool(name="stg", bufs=3) as stg, \
             tc.tile_pool(name="pg", bufs=4, space="PSUM") as pg:
            bidt_t = wsp.tile([128, 60], F32, tag="bidt", name="bidt")
            nc.sync.dma_start(bidt_t[:], bid_d[:])
            iotat = wsp.tile([128, 256], FP16, tag="iota", name="iotat")
            nc.sync.dma_start(iotat[:], iota_d[:])

            s_t = []
            bcol = {"n": 0}

            def sbuild(dst):
                nc.vector.tensor_scalar(
                    out=dst, in0=iotat[:],
                    scalar1=bidt_t[:, bcol["n"]:bcol["n"] + 1], scalar2=None,
                    op0=mybir.AluOpType.is_equal)
                bcol["n"] += 1

            for g, (rows, cols) in enumerate([(512, 256), (256, 1024),
                                              (128, 4096), (128, 8192)]):
                tiles = []
                for kt in range(rows // 128):
                    t = wsp.tile([128, cols], FP16, tag=f"s{g}_{kt}",
                                 name=f"s{g}_{kt}")
                    tiles.append(t)
                s_t.append(tiles)
            # column order must match host bidt layout
            for kt in range(4):
                sbuild(s_t[0][kt][:])
            for rho in range(4):
                for kt in range(2):
                    sbuild(s_t[1][kt][:, rho * 256:(rho + 1) * 256])
            for rho in range(16):
                sbuild(s_t[2][0][:, rho * 256:(rho + 1) * 256])
            for pair in range(32):
                sbuild(s_t[3][0][:, pair * 256:(pair + 1) * 256])
            w_t = []
            for g, rows in enumerate([512, 256, 128, 128]):
                tiles = []
                for kt in range(rows // 128):
                    t = wsp.tile([128, gtot[g]], FP16, tag=f"w{g}_{kt}",
                                 name=f"w{g}_{kt}")
                    nc.sync.dma_start(t[:], w_d[g][128 * kt:128 * (kt + 1), :])
                    tiles.append(t)
                w_t.append(tiles)

            stage = None
            pspair = {"t": None}
            for (g, rho, nrho, P, boff, goffg, coff, cw, j) in chunks:
                slot = j % STAGE_SLOTS
                if slot == 0:
                    stage = stg.tile([128, 256 * STAGE_SLOTS], FP16,
                                     tag="stage", name="stage")
                col0 = (rho if nrho == 1 else rho // 2) * 256
                ktn = KT_N[g]
                half = j % 2
                if half == 0:
                    pspair["t"] = pg.tile([128, 512], F32, tag="pg", name="ps")
                ps = pspair["t"]
                for kt in range(ktn):
                    nc.tensor.matmul(
                        ps[:cw, 256 * half:256 * half + 256],
                        w_t[g][kt][:, goffg + coff:goffg + coff + cw],
                        s_t[g][kt][:, col0:col0 + 256],
                        start=(kt == 0),
                        stop=(kt == ktn - 1),
                        skip_group_check=True,
                    )
                if half == 1 or j == NCH - 1:
                    wfull = 256 * (half + 1)
                    alt_copy(stage[:, 256 * (slot - half):
                                   256 * (slot - half) + wfull],
                             ps[:, :wfull])
                if slot == STAGE_SLOTS - 1 or j == NCH - 1:
                    nsl = slot + 1
                    jt = j - slot
                    dview = td_d[128 * jt:128 * (jt + nsl), :].rearrange(
                        "(s p) i -> p s i", p=128)
                    sview = stage[:, :256 * nsl].rearrange(
                        "p (s i) -> p s i", s=nsl)
                    nc.gpsimd.dma_start(dview, sview)

        # ---- phase 2+3: sorted gathers + transposes + GEMM1 + combine ----
        MULT = mybir.AluOpType.mult
        ADD = mybir.AluOpType.add
        with tc.tile_pool(name="gtp", bufs=12) as gtp, \
             tc.tile_pool(name="scrp", bufs=2) as scrp, \
             tc.tile_pool(name="pt", bufs=2, space="PSUM") as pt, \
             tc.tile_pool(name="pz", bufs=3, space="PSUM") as pz:
            for c in range(64):
                gt = gtp.tile([128, 256], FP16, tag="gt", name="gt")
                nc.gpsimd.indirect_dma_start(
                    out=gt[:],
                    out_offset=None,
                    in_=td_d[:],
                    in_offset=IndirectOffsetOnAxis(ap=idxt[:, c:c + 1], axis=0),
                )
                for ih in range(2):
                    tp = pt.tile([128, 128], FP16, tag="tp", name="tp")
                    nc.tensor.transpose(
                        tp[:], gt[:, 128 * ih:128 * (ih + 1)], ident[:])
                    # fold the 0.1 output scale into T here (free);
                    # Scalar-only: DVE is loaded with early combine work
                    nc.scalar.mul(Tb[ih][:, 128 * c:128 * (c + 1)], tp[:], 0.1)

            # combine per Z tile: Scalar evacuates PSUM->SBUF fp16, then DVE
            # does ONE 512-wide mult against the host-shipped [x|x] pair tile
            # (2x mode), ONE 2x fold-add halving each o-segment, and the 1x
            # reduce over the remaining 128-spans.
            for ncp in range(8):
                for bt in range(16):
                    zt = pz.tile([128, 1024], F32, tag="z", name="zt")
                    for half in range(2):
                        nci = 2 * ncp + half
                        for ih in range(2):
                            nc.tensor.matmul(
                                zt[:, 512 * half:512 * (half + 1)],
                                xtt[ih][:, 128 * bt:128 * (bt + 1)],
                                Tb[ih][:, 512 * nci:512 * (nci + 1)],
                                start=(ih == 0),
                                stop=(ih == 1),
                            )
                    zb = scrp.tile([128, 1024], FP16, tag="zb", name="zb",
                                   bufs=6)
                    nc.scalar.copy(zb[:], zt[:])
                    zm = scrp.tile([128, 1024], FP16, tag="zm", name="zm",
                                   bufs=4)
                    xq = xbt[bt][:].rearrange(
                        "p (s f) -> p s f", s=1).broadcast_to([128, 4, 256])
                    nc.vector.tensor_tensor(
                        out=zm[:].rearrange("p (s f) -> p s f", s=4),
                        in0=zb[:].rearrange("p (s f) -> p s f", s=4),
                        in1=xq, op=MULT)
                    zf = scrp.tile([128, 512], FP16, tag="zf", name="zf",
                                   bufs=4)
                    zm4 = zm[:].rearrange("p (s f) -> p s f", s=4)
                    nc.vector.tensor_tensor(
                        out=zf[:].rearrange("p (s f) -> p s f", s=4),
                        in0=zm4[:, :, 0:128], in1=zm4[:, :, 128:256], op=ADD)
                    nc.vector.tensor_reduce(
                        out=outsb[bt][:, 4 * ncp:4 * ncp + 4],
                        in_=zf[:].rearrange("p (s f) -> p s f", s=4),
                        op=ADD,
                        axis=mybir.AxisListType.X,
                    )
            for bt in range(16):
                nc.sync.dma_start(out_d[128 * bt:128 * (bt + 1), :], outsb[bt][:])

    nc.compile()
    return nc


_NC_CACHE = None
_NC_KEY = None


def _make_in_maps(x, layout, s_arrs, w_percore, idx_percore):
    x = np.ascontiguousarray(np.asarray(x, dtype=np.float32))
    xb = np.ascontiguousarray(x.astype(np.float16))
    xt = np.ascontiguousarray(x.T.astype(np.float16))
    in_maps = []
    for c in range(NCORES):
        in_maps.append({
            "bidt": s_arrs[0], "iota": s_arrs[1],
            "w0": w_percore[c][0], "w1": w_percore[c][1],
            "w2": w_percore[c][2], "w3": w_percore[c][3],
            "xt": xt, "xb": xb, "idx": idx_percore[c],
        })
    return in_maps


def kernel(x, W0, W1, W2, W3, bid0, bid1, bid2, bid3, W_invperm, **_unused):
    global _NC_CACHE, _NC_KEY
    prep = _host_prep(W0, W1, W2, W3, bid0, bid1, bid2, bid3, W_invperm)
    layout = prep[0]
    key = hash(layout)
    if _NC_CACHE is None or _NC_KEY != key:
        _NC_CACHE = _build_nc(layout)
        _NC_KEY = key
    nc = _NC_CACHE

    in_maps = _make_in_maps(x, *prep)
    res = run_bass_kernel_spmd(nc, in_maps, core_ids=list(range(NCORES)))
    out = np.concatenate([res.results[c]["out"] for c in range(NCORES)], axis=1)
    return np.ascontiguousarray(out.astype(np.float32))
